# revision 7
# baseline (speedup 1.0000x reference)
"""Trainium2 Bass kernel for nn_CustomModel_7378753814838.

Math (reference):
    a = x1.reshape(N,R,F); b = x2.reshape(N,R,F)
    d2[k,n,i,j] = ||a[n,i] - b[n,j] - m_k||^2
    kv = exp(-d2 / (2*sigma_k^2))
    out = sum_k w_k * softmax_j(kv[k])           w = softmax(1/sigma_params^2)

Key identities used:
    d2 = ||(a - m_k) - b||^2 = sa'2_k[i] + sb2[j] - 2*dot'_k[i,j]
      sa'2_k[i] = sum_f (a[i,f]-m_k)^2 ;  sb2[j] = sum_f b[j,f]^2
      dot'_k    = (A - m_k) @ B^T
    kv = exp(SCALE_k * (psum)) with psum = -2dot' + sb2[j] (PE matmuls),
      + per-partition bias SCALE_k*sa'2[i] on the ACT exp.  SCALE_k=-1/(2 s^2)
    Kernels with negligible weight w_k (< 1e-12) are dropped host-side.

Sharding: data-parallel over N across 8 cores (16 samples each).
"""

import numpy as np

N, R, F, K = 128, 128, 128, 4
NCORES = 8
NP = N // NCORES  # samples per core


def _build_nc(sigmas, means, sigma_params):
    from contextlib import ExitStack

    import concourse.bacc as bacc
    import concourse.tile as tile
    from concourse import mybir

    f32 = mybir.dt.float32
    ALU = mybir.AluOpType
    ACTF = mybir.ActivationFunctionType

    # ---- host-side scalar math (f64) ----
    sig = np.asarray(sigmas, dtype=np.float64)
    mu = np.asarray(means, dtype=np.float64)
    sp = np.asarray(sigma_params, dtype=np.float64)
    logits = 1.0 / (sp * sp)
    e = np.exp(logits - logits.max())
    w = e / e.sum()
    KS = [k for k in range(K) if w[k] > 1e-12]
    SCALE = [-1.0 / (2.0 * sig[k] * sig[k]) for k in range(K)]

    nc = bacc.Bacc(
        "TRN2",
        target_bir_lowering=False,
        debug=False,
        enable_asserts=False,
        num_devices=NCORES,
    )
    x1 = nc.dram_tensor("x1", [NP, R * F], f32, kind="ExternalInput").ap()
    x2 = nc.dram_tensor("x2", [NP, R * F], f32, kind="ExternalInput").ap()
    y = nc.dram_tensor("y", [NP, R, R], f32, kind="ExternalOutput").ap()

    id_p1_d = nc.inline_tensor(np.eye(R).astype(np.float32), name="id_p1").ap()

    A_src = x1.rearrange("n (i f) -> i n f", i=R)  # [128, NP, 128]
    B_src = x2.rearrange("n (j f) -> j n f", j=R)
    y_dst = y.rearrange("n i j -> i n j")  # [128, NP, 128]

    NG = NP // 4  # groups of 4 samples

    with ExitStack() as ctx:
        tc = ctx.enter_context(tile.TileContext(nc))
        singles = ctx.enter_context(tc.tile_pool(name="singles", bufs=1))
        bigs = ctx.enter_context(tc.tile_pool(name="bigs", bufs=1))
        kbig = ctx.enter_context(tc.tile_pool(name="kbig", bufs=1))
        stats = ctx.enter_context(tc.tile_pool(name="stats", bufs=1))
        trash = ctx.enter_context(tc.tile_pool(name="trash", bufs=4))
        psA = ctx.enter_context(tc.tile_pool(name="psA", bufs=2, space="PSUM"))
        psB = ctx.enter_context(tc.tile_pool(name="psB", bufs=2, space="PSUM"))
        psG = ctx.enter_context(tc.tile_pool(name="psG", bufs=2, space="PSUM"))
        psS = ctx.enter_context(tc.tile_pool(name="psS", bufs=2, space="PSUM"))

        # constants
        id_p1 = singles.tile([R, R], f32)
        nc.sync.dma_start(id_p1[:], id_p1_d)
        ones_r = singles.tile([1, R], f32)
        nc.vector.memset(ones_r[:], 1.0)

        # inputs, 4-sample chunks for pipelining
        A = bigs.tile([R, NP, F], f32, tag="A")
        B = bigs.tile([R, NP, F], f32, tag="B")
        for g in range(NG):
            s = slice(4 * g, 4 * g + 4)
            nc.sync.dma_start(A[:, s, :], A_src[:, s, :])
            nc.sync.dma_start(B[:, s, :], B_src[:, s, :])

        # ---- stats ----
        sa = stats.tile([R, NP], f32, tag="sa")
        sa2 = stats.tile([R, NP], f32, tag="sa2")
        sb2 = stats.tile([R, NP], f32, tag="sb2")
        for g in range(NG):
            s = slice(4 * g, 4 * g + 4)
            nc.vector.tensor_reduce(
                sa[:, s], A[:, s, :], axis=mybir.AxisListType.X, op=ALU.add
            )
            sq = trash.tile([R, 4, F], f32, tag="sq")
            nc.gpsimd.tensor_mul(sq[:], A[:, s, :], A[:, s, :])
            nc.vector.tensor_reduce(
                sa2[:, s], sq[:], axis=mybir.AxisListType.X, op=ALU.add
            )
            sqb = trash.tile([R, 4, F], f32, tag="sq")
            nc.gpsimd.tensor_mul(sqb[:], B[:, s, :], B[:, s, :])
            nc.vector.tensor_reduce(
                sb2[:, s], sqb[:], axis=mybir.AxisListType.X, op=ALU.add
            )

        # sb2 -> row layout [1, NP*128] via PE transpose + copy + sbuf-dma
        rex = singles.tile([1, NP, R], f32)
        for g in range(NG):
            s = slice(4 * g, 4 * g + 4)
            pT = psS.tile([4, R], f32, tag="pT")
            nc.tensor.transpose(pT[:], sb2[:, s], id_p1[:])
            sb2T = trash.tile([4, R], f32, tag="sb2T")
            nc.vector.tensor_copy(sb2T[:], pT[:])
            nc.sync.dma_start(rex[:, s, :], sb2T[:])

        # per-kernel per-partition bias columns: ubias_k = SCALE*(sa2 - 2 m sa + F m^2)
        ub = {}
        for k in KS:
            m = float(mu[k])
            sc = float(SCALE[k])
            pre = stats.tile([R, NP], f32, tag=f"pre{k}")
            nc.vector.scalar_tensor_tensor(
                pre[:], sa[:], -2.0 * m, sa2[:], op0=ALU.mult, op1=ALU.add
            )
            ubk = stats.tile([R, NP], f32, tag=f"ub{k}")
            nc.vector.tensor_scalar(
                ubk[:], pre[:], sc, sc * F * m * m, op0=ALU.mult, op1=ALU.add
            )
            ub[k] = ubk

        # ---- transposes of A (scaled by -2) and B ----
        BT = bigs.tile([R, NP, F], f32, tag="BT")
        ATs = {
            k: kbig.tile([R, NP, F], f32, tag=f"ATs{k}", name=f"ATs{k}") for k in KS
        }
        for g in range(NG):
            s = slice(4 * g, 4 * g + 4)
            pA = psA.tile([R, 4, F], f32, tag="pA")
            pB = psB.tile([R, 4, F], f32, tag="pB")
            for q in range(4):
                nc.tensor.transpose(pA[:, q, :], A[:, 4 * g + q, :], id_p1[:])
                nc.tensor.transpose(pB[:, q, :], B[:, 4 * g + q, :], id_p1[:])
            nc.vector.tensor_copy(BT[:, s, :], pB[:])
            for k in KS:
                # ATs = -2*A^T + 2m  ( = -2*(A-m)^T ; transpose-mode ignores
                # the identity's values, so the scale lives here)
                nc.vector.tensor_scalar(
                    ATs[k][:, s, :],
                    pA[:],
                    -2.0,
                    2.0 * float(mu[k]),
                    op0=ALU.mult,
                    op1=ALU.add,
                )

        # ---- per-kernel main pipeline ----
        OUT = bigs.tile([R, NP, F], f32, tag="OUT")
        for ki, k in enumerate(KS):
            sc = float(SCALE[k])
            KV = kbig.tile([R, NP, F], f32, tag="KV")
            E = kbig.tile([R, NP, F], f32, tag="E")
            for g in range(NG):
                pG = psG.tile([R, 4, F], f32, tag="pG")
                for q in range(4):
                    n = 4 * g + q
                    nc.tensor.matmul(
                        pG[:, q, :],
                        lhsT=ATs[k][:, n, :],
                        rhs=BT[:, n, :],
                        start=True,
                        stop=False,
                    )
                    nc.tensor.matmul(
                        pG[:, q, :],
                        lhsT=ones_r[:],
                        rhs=rex[:, n, :],
                        start=False,
                        stop=True,
                    )
                for q in range(4):
                    n = 4 * g + q
                    nc.scalar.activation(
                        KV[:, n, :],
                        pG[:, q, :],
                        ACTF.Exp,
                        bias=ub[k][:, n : n + 1],
                        scale=sc,
                    )
            # second exp, in halves of 8 samples
            for h in range(2):
                s = slice(8 * h, 8 * h + 8)
                nc.scalar.activation(E[:, s, :], KV[:, s, :], ACTF.Exp)
            # softmax denominator, reciprocal, weight
            scol = stats.tile([R, NP], f32, tag="scol")
            for h in range(2):
                s = slice(8 * h, 8 * h + 8)
                nc.vector.tensor_reduce(
                    scol[:, s], E[:, s, :], axis=mybir.AxisListType.X, op=ALU.add
                )
            qcol = stats.tile([R, NP], f32, tag="qcol")
            nc.vector.reciprocal(qcol[:], scol[:])
            if w[k] != 1.0:
                nc.vector.tensor_scalar(
                    qcol[:], qcol[:], float(w[k]), None, op0=ALU.mult
                )
            last = ki == len(KS) - 1
            for g in range(NG):
                s = slice(4 * g, 4 * g + 4)
                for q in range(4):
                    n = 4 * g + q
                    if ki == 0:
                        nc.vector.tensor_scalar(
                            OUT[:, n, :],
                            E[:, n, :],
                            qcol[:, n : n + 1],
                            None,
                            op0=ALU.mult,
                        )
                    else:
                        nc.vector.scalar_tensor_tensor(
                            OUT[:, n, :],
                            E[:, n, :],
                            qcol[:, n : n + 1],
                            OUT[:, n, :],
                            op0=ALU.mult,
                            op1=ALU.add,
                        )
                if last:
                    import os

                    dbg = os.environ.get("KDBG", "")
                    src = {"kv": KV, "e": E, "out": OUT}.get(dbg, OUT)
                    nc.sync.dma_start(y_dst[:, s, :], src[:, s, :])

    nc.compile()
    return nc


_CACHE = {}


def _get_nc(key, sigmas, means, sigma_params):
    if key not in _CACHE:
        _CACHE[key] = _build_nc(sigmas, means, sigma_params)
    return _CACHE[key]


def run(x1, x2, sigmas, means, sigma_params, trace=False, **rk):
    from concourse.bass_utils import run_bass_kernel_spmd

    key = (sigmas.tobytes(), means.tobytes(), sigma_params.tobytes())
    nc = _get_nc(key, sigmas, means, sigma_params)

    x1 = np.ascontiguousarray(x1, dtype=np.float32)
    x2 = np.ascontiguousarray(x2, dtype=np.float32)
    in_maps = []
    for c in range(NCORES):
        s = slice(c * NP, (c + 1) * NP)
        in_maps.append({"x1": x1[s], "x2": x2[s]})
    res = run_bass_kernel_spmd(
        nc, in_maps, core_ids=list(range(NCORES)), trace=trace, **rk
    )
    out = np.concatenate([r["y"] for r in res.results], axis=0)
    return out, res


def kernel(x1, x2, sigmas, means, sigma_params):
    out, _ = run(x1, x2, sigmas, means, sigma_params, trace=False)
    return out


# revision 10
# speedup vs baseline: 1.3227x; 1.3227x over previous
"""Trainium2 Bass kernel for nn_CustomModel_7378753814838.

Math (reference):
    a = x1.reshape(N,R,F); b = x2.reshape(N,R,F)
    d2[k,n,i,j] = ||a[n,i] - b[n,j] - m_k||^2
    kv = exp(-d2 / (2*sigma_k^2))
    out = sum_k w_k * softmax_j(kv[k])           w = softmax(1/sigma_params^2)

Key identities used:
    d2 = ||(a - m_k) - b||^2 = sa'2_k[i] + sb2[j] - 2*dot'_k[i,j]
      sa'2_k[i] = sum_f (a[i,f]-m_k)^2 ;  sb2[j] = sum_f b[j,f]^2
      dot'_k    = (A - m_k) @ B^T
    kv = exp(SCALE_k * (psum)) with psum = -2dot' + sb2[j] (PE matmuls),
      + per-partition bias SCALE_k*sa'2[i] on the ACT exp.  SCALE_k=-1/(2 s^2)
    Kernels with negligible weight w_k (< 1e-12) are dropped host-side.

Sharding: data-parallel over N across 8 cores (16 samples each).
"""

import numpy as np

N, R, F, K = 128, 128, 128, 4
NCORES = 8
NP = N // NCORES  # samples per core


def _build_nc(sigmas, means, sigma_params):
    from contextlib import ExitStack

    import concourse.bacc as bacc
    import concourse.tile as tile
    from concourse import mybir

    f32 = mybir.dt.float32
    ALU = mybir.AluOpType
    ACTF = mybir.ActivationFunctionType

    # ---- host-side scalar math (f64) ----
    sig = np.asarray(sigmas, dtype=np.float64)
    mu = np.asarray(means, dtype=np.float64)
    sp = np.asarray(sigma_params, dtype=np.float64)
    logits = 1.0 / (sp * sp)
    e = np.exp(logits - logits.max())
    w = e / e.sum()
    KS = [k for k in range(K) if w[k] > 1e-12]
    SCALE = [-1.0 / (2.0 * sig[k] * sig[k]) for k in range(K)]

    nc = bacc.Bacc(
        "TRN2",
        target_bir_lowering=False,
        debug=False,
        enable_asserts=False,
        num_devices=NCORES,
    )
    x1 = nc.dram_tensor("x1", [NP, R * F], f32, kind="ExternalInput").ap()
    x2 = nc.dram_tensor("x2", [NP, R * F], f32, kind="ExternalInput").ap()
    y = nc.dram_tensor("y", [NP, R, R], f32, kind="ExternalOutput").ap()

    id_p1_d = nc.inline_tensor(np.eye(R).astype(np.float32), name="id_p1").ap()

    A_src = x1.rearrange("n (i f) -> i n f", i=R)  # [128, NP, 128]
    B_src = x2.rearrange("n (j f) -> j n f", j=R)
    y_dst = y.rearrange("n i j -> i n j")  # [128, NP, 128]

    NG = NP // 4  # groups of 4 samples

    with ExitStack() as ctx:
        tc = ctx.enter_context(tile.TileContext(nc))
        singles = ctx.enter_context(tc.tile_pool(name="singles", bufs=1))
        bigs = ctx.enter_context(tc.tile_pool(name="bigs", bufs=1))
        kbig = ctx.enter_context(tc.tile_pool(name="kbig", bufs=1))
        stats = ctx.enter_context(tc.tile_pool(name="stats", bufs=1))
        trash = ctx.enter_context(tc.tile_pool(name="trash", bufs=4))
        psA = ctx.enter_context(tc.tile_pool(name="psA", bufs=2, space="PSUM"))
        psB = ctx.enter_context(tc.tile_pool(name="psB", bufs=2, space="PSUM"))
        psG = ctx.enter_context(tc.tile_pool(name="psG", bufs=2, space="PSUM"))
        psS = ctx.enter_context(tc.tile_pool(name="psS", bufs=2, space="PSUM"))

        # constants
        id_p1 = singles.tile([R, R], f32)
        nc.sync.dma_start(id_p1[:], id_p1_d)
        ones_r = singles.tile([1, R], f32)
        nc.vector.memset(ones_r[:], 1.0)

        # inputs, 4-sample chunks for pipelining
        A = bigs.tile([R, NP, F], f32, tag="A")
        B = bigs.tile([R, NP, F], f32, tag="B")
        for g in range(NG):
            s = slice(4 * g, 4 * g + 4)
            nc.sync.dma_start(A[:, s, :], A_src[:, s, :])
            nc.sync.dma_start(B[:, s, :], B_src[:, s, :])

        # ---- stats ----
        sa = stats.tile([R, NP], f32, tag="sa")
        sa2 = stats.tile([R, NP], f32, tag="sa2")
        sb2 = stats.tile([R, NP], f32, tag="sb2")
        for g in range(NG):
            s = slice(4 * g, 4 * g + 4)
            nc.vector.tensor_reduce(
                sa[:, s], A[:, s, :], axis=mybir.AxisListType.X, op=ALU.add
            )
            sq = trash.tile([R, 4, F], f32, tag="sq")
            nc.gpsimd.tensor_mul(sq[:], A[:, s, :], A[:, s, :])
            nc.vector.tensor_reduce(
                sa2[:, s], sq[:], axis=mybir.AxisListType.X, op=ALU.add
            )
            sqb = trash.tile([R, 4, F], f32, tag="sq")
            nc.gpsimd.tensor_mul(sqb[:], B[:, s, :], B[:, s, :])
            nc.vector.tensor_reduce(
                sb2[:, s], sqb[:], axis=mybir.AxisListType.X, op=ALU.add
            )

        # sb2 -> row layout [1, NP*128] via PE transpose + copy + sbuf-dma
        rex = singles.tile([1, NP, R], f32)
        for g in range(NG):
            s = slice(4 * g, 4 * g + 4)
            pT = psS.tile([4, R], f32, tag="pT")
            nc.tensor.transpose(pT[:], sb2[:, s], id_p1[:])
            sb2T = trash.tile([4, R], f32, tag="sb2T")
            nc.vector.tensor_copy(sb2T[:], pT[:])
            nc.sync.dma_start(rex[:, s, :], sb2T[:])

        # per-kernel per-partition bias columns: ubias_k = SCALE*(sa2 - 2 m sa + F m^2)
        # computed per-group so exp1 of group g doesn't wait on other groups
        ub = {}
        for k in KS:
            m = float(mu[k])
            sc = float(SCALE[k])
            ubk = stats.tile([R, NP], f32, tag=f"ub{k}", name=f"ub{k}")
            for g in range(NG):
                s = slice(4 * g, 4 * g + 4)
                pre = trash.tile([R, 4], f32, tag="pre")
                nc.vector.scalar_tensor_tensor(
                    pre[:], sa[:, s], -2.0 * m, sa2[:, s], op0=ALU.mult, op1=ALU.add
                )
                nc.vector.tensor_scalar(
                    ubk[:, s], pre[:], sc, sc * F * m * m, op0=ALU.mult, op1=ALU.add
                )
            ub[k] = ubk

        # ---- transposes of A (scaled by -2) and B ----
        BT = bigs.tile([R, NP, F], f32, tag="BT")
        ATs = {
            k: kbig.tile([R, NP, F], f32, tag=f"ATs{k}", name=f"ATs{k}") for k in KS
        }
        for g in range(NG):
            s = slice(4 * g, 4 * g + 4)
            pA = psA.tile([R, 4, F], f32, tag="pA")
            pB = psB.tile([R, 4, F], f32, tag="pB")
            for q in range(4):
                nc.tensor.transpose(pA[:, q, :], A[:, 4 * g + q, :], id_p1[:])
                nc.tensor.transpose(pB[:, q, :], B[:, 4 * g + q, :], id_p1[:])
            if g % 2 == 0:
                nc.vector.tensor_copy(BT[:, s, :], pB[:])
            else:
                nc.scalar.copy(BT[:, s, :], pB[:])
            for k in KS:
                # ATs = -2*A^T + 2m  ( = -2*(A-m)^T ; transpose-mode ignores
                # the identity's values, so the scale lives here)
                nc.vector.tensor_scalar(
                    ATs[k][:, s, :],
                    pA[:],
                    -2.0,
                    2.0 * float(mu[k]),
                    op0=ALU.mult,
                    op1=ALU.add,
                )

        # ---- per-kernel main pipeline (fully per-group for pipelining) ----
        OUT = bigs.tile([R, NP, F], f32, tag="OUT")
        for ki, k in enumerate(KS):
            sc = float(SCALE[k])
            KV = kbig.tile([R, NP, F], f32, tag="KV")
            E = kbig.tile([R, NP, F], f32, tag="E")
            last = ki == len(KS) - 1
            for g in range(NG):
                s = slice(4 * g, 4 * g + 4)
                pG = psG.tile([R, 4, F], f32, tag="pG")
                for q in range(4):
                    n = 4 * g + q
                    # q==0 clears the whole bank's has_written bits; later
                    # matmuls overwrite-where-unset / accumulate-where-set
                    nc.tensor.matmul(
                        pG[:, q, :],
                        lhsT=ATs[k][:, n, :],
                        rhs=BT[:, n, :],
                        start=(q == 0),
                        stop=False,
                    )
                # one whole-bank matmul adds sb2[j] to all 4 samples
                nc.tensor.matmul(
                    pG[:, :, :],
                    lhsT=ones_r[:],
                    rhs=rex[:, s, :],
                    start=False,
                    stop=True,
                )
                for q in range(4):
                    n = 4 * g + q
                    nc.scalar.activation(
                        KV[:, n, :],
                        pG[:, q, :],
                        ACTF.Exp,
                        bias=ub[k][:, n : n + 1],
                        scale=sc,
                    )
                nc.scalar.activation(E[:, s, :], KV[:, s, :], ACTF.Exp)
                scol = trash.tile([R, 4], f32, tag="scol")
                nc.vector.tensor_reduce(
                    scol[:], E[:, s, :], axis=mybir.AxisListType.X, op=ALU.add
                )
                qcol = trash.tile([R, 4], f32, tag="qcol")
                nc.vector.reciprocal(qcol[:], scol[:])
                if w[k] != 1.0:
                    nc.vector.tensor_scalar(
                        qcol[:], qcol[:], float(w[k]), None, op0=ALU.mult
                    )
                for q in range(4):
                    n = 4 * g + q
                    if ki == 0:
                        nc.vector.tensor_scalar(
                            OUT[:, n, :],
                            E[:, n, :],
                            qcol[:, q : q + 1],
                            None,
                            op0=ALU.mult,
                        )
                    else:
                        nc.vector.scalar_tensor_tensor(
                            OUT[:, n, :],
                            E[:, n, :],
                            qcol[:, q : q + 1],
                            OUT[:, n, :],
                            op0=ALU.mult,
                            op1=ALU.add,
                        )
                if last:
                    nc.sync.dma_start(y_dst[:, s, :], OUT[:, s, :])

    nc.compile()
    return nc


_CACHE = {}


def _get_nc(key, sigmas, means, sigma_params):
    if key not in _CACHE:
        _CACHE[key] = _build_nc(sigmas, means, sigma_params)
    return _CACHE[key]


def run(x1, x2, sigmas, means, sigma_params, trace=False, **rk):
    from concourse.bass_utils import run_bass_kernel_spmd

    key = (sigmas.tobytes(), means.tobytes(), sigma_params.tobytes())
    nc = _get_nc(key, sigmas, means, sigma_params)

    x1 = np.ascontiguousarray(x1, dtype=np.float32)
    x2 = np.ascontiguousarray(x2, dtype=np.float32)
    in_maps = []
    for c in range(NCORES):
        s = slice(c * NP, (c + 1) * NP)
        in_maps.append({"x1": x1[s], "x2": x2[s]})
    res = run_bass_kernel_spmd(
        nc, in_maps, core_ids=list(range(NCORES)), trace=trace, **rk
    )
    out = np.concatenate([r["y"] for r in res.results], axis=0)
    return out, res


def kernel(x1, x2, sigmas, means, sigma_params):
    out, _ = run(x1, x2, sigmas, means, sigma_params, trace=False)
    return out
